# revision 7
# baseline (speedup 1.0000x reference)
"""Multi-head attention with RoPE on 8 Trainium2 NeuronCores.

Sharding: core i handles batch (i // 4) and heads 4*(i % 4) .. 4*(i % 4)+3
(data parallel on B=2, tensor parallel on the 16 heads / the matching
column blocks of Wq/Wk/Wv and row block of Wo). Each core computes its
partial output projection [S, D] (transposed on device); the host sums
the 4 tensor-parallel partials per batch and adds bo.

Per-core device pipeline (all matmuls float32r):
  1. x [2048,1024] DMA'd natural, PE-transposed to xT [din, s] in SBUF.
  2. qT/kT = Wq/Wk.T @ xT (+bias via ACT), RoPE applied on DVE with
     host-precomputed cos / sign-folded-sin tables in the transposed
     layout; v = xT.T @ Wv natural (+bias via K=1 ones-matmul), stored
     ones-augmented [s, 65] per head.
  3. Per head pair, per 512-wide q tile, per 128-wide k chunk:
     scoresT = kT.T-block @ qT (two heads packed in the 128x128 array via
     row tiling at partitions 0/64, separate PSUM banks), ACT exp with
     scale 1/sqrt(dk), then AV: v_aug.T @ expT accumulates [65, 512] —
     row 64 is the softmax denominator (ones column).
  4. 1/Z via DVE reciprocal, broadcast across 64 partitions with a K=1
     ones-matmul, context normalized on DVE into ctxT (f32r).
  5. out_T = Wo_slice.T @ ctxT -> [1024, 2048] partial, DMA'd out.
"""
import sys

if '/opt/trn_rl_repo' not in sys.path:
    sys.path.insert(0, '/opt/trn_rl_repo')

import numpy as np

import concourse.bass as bass
import concourse.mybir as mybir
from concourse.tile import TileContext
from concourse.masks import make_identity
from concourse.bass_utils import run_bass_kernel_spmd

F32 = mybir.dt.float32
F32R = mybir.dt.float32r

B, S, D = 2, 2048, 1024
H, DK = 16, 64
TP = 4                  # tensor-parallel ways (head groups)
HPC = H // TP           # heads per core = 4
DH = HPC * DK           # per-core projection width = 256
NT = 512                # moving-operand tile (f32r max)
SC = S // 128           # 16 s-chunks
KC = D // 128           # 8 contraction chunks over D
MC = DH // 128          # 2 dout chunks per core
NQT = S // NT           # 4 q tiles
SH = 2                  # s-halves for the xT buffer

_ENGINES = {
    mybir.EngineType.PE,
    mybir.EngineType.Activation,
    mybir.EngineType.DVE,
    mybir.EngineType.SP,
    mybir.EngineType.Pool,
}


def _split_multi_waits(nc):
    """This walrus build accepts one sync-wait per engine instruction;
    move Tile's extra waits onto preceding same-engine NoOps."""
    for fn in nc.m.functions:
        for blk in fn.blocks:
            insts = blk.instructions
            i = 0
            while i < len(insts):
                inst = insts[i]
                si = getattr(inst, "sync_info", None)
                if (
                    si is not None
                    and len(si.on_wait) > 1
                    and getattr(inst, "engine", None) in _ENGINES
                ):
                    extra = si.on_wait[:-1]
                    si.on_wait = si.on_wait[-1:]
                    nops = [
                        mybir.InstNoOp(
                            name=nc.get_next_instruction_name(),
                            engine=inst.engine,
                            bass_nofuse=True,
                            sync_info=mybir.SyncInfo(on_wait=[w], on_update=[]),
                        )
                        for w in extra
                    ]
                    insts[i:i] = nops
                    i += len(nops)
                i += 1


def _build_core_body(nc, tens, sb_main, tc):
    """Emit one full attention pass. `tens` maps dram tensor name -> handle."""
    x_d, wq_d, wk_d, wv_d, wo_d = (tens[k] for k in ("x_b", "wq", "wk", "wv", "wo"))
    bq_d, bk_d, bv_d = tens["bq"], tens["bk"], tens["bv"]
    cos_d, sin_d, ones_d = tens["cos_t"], tens["sin_s"], tens["ones"]
    out_d = tens["outT"]

    with (
        tc.tile_pool(name="sb_p1", bufs=1) as sb_p1,
        tc.tile_pool(name="ps1", bufs=1, space="PSUM") as ps,
    ):
        # ---- constants / weights ----
        ident = sb_main.tile([128, 128], F32, tag="ident")
        make_identity(nc, ident[:])
        ones_f = sb_main.tile([1, 128], F32, tag="ones_f")
        nc.sync.dma_start(ones_f[:], ones_d.ap())
        ones_r = sb_main.tile([1, 128], F32R, tag="ones_r")
        nc.scalar.copy(ones_r[:], ones_f[:])

        wq_t = sb_main.tile([128, KC, DH], F32R, tag="wq")
        wk_t = sb_main.tile([128, KC, DH], F32R, tag="wk")
        wv_t = sb_main.tile([128, KC, DH], F32R, tag="wv")
        nc.gpsimd.dma_start(wq_t[:], wq_d.ap().rearrange("(c p) m -> p c m", p=128))
        nc.gpsimd.dma_start(wk_t[:], wk_d.ap().rearrange("(c p) m -> p c m", p=128))
        nc.gpsimd.dma_start(wv_t[:], wv_d.ap().rearrange("(c p) m -> p c m", p=128))
        wo_t = sb_main.tile([128, MC, D], F32R, tag="wo")
        nc.gpsimd.dma_start(wo_t[:], wo_d.ap().rearrange("(c p) m -> p c m", p=128))

        bq_t = sb_main.tile([128, MC], F32, tag="bq")
        bk_t = sb_main.tile([128, MC], F32, tag="bk")
        nc.sync.dma_start(bq_t[:], bq_d.ap().rearrange("(c p) -> p c", p=128))
        nc.sync.dma_start(bk_t[:], bk_d.ap().rearrange("(c p) -> p c", p=128))
        bqs_t = sb_main.tile([128, MC], F32, tag="bqs")
        bks_t = sb_main.tile([128, MC], F32, tag="bks")
        nc.sync.dma_start(bqs_t[:], tens["bq_sh"].ap().rearrange("(c p) -> p c", p=128))
        nc.sync.dma_start(bks_t[:], tens["bk_sh"].ap().rearrange("(c p) -> p c", p=128))
        bv_t = sb_main.tile([1, DH], F32R, tag="bv")
        nc.gpsimd.dma_start(bv_t[:], bv_d.ap())

        cos_t = sb_p1.tile([128, S], F32, tag="cos")
        sin_t = sb_p1.tile([128, S], F32, tag="sin")
        nc.sync.dma_start(cos_t[:], cos_d.ap())
        nc.sync.dma_start(sin_t[:], sin_d.ap())

        # ---- persistent activations ----
        qT = sb_main.tile([128, MC, S], F32R, tag="qT")
        kT = sb_main.tile([128, MC, S], F32R, tag="kT")
        v_aug = sb_main.tile([128, SC, HPC, DK + 1], F32R, tag="v_aug")
        ctx = sb_main.tile([128, MC, S], F32R, tag="ctx")

        # ones column of v_aug: func(0*in + 1) = 1.0 for every (sc, h)
        with nc.allow_low_precision(reason="f32r ones column"):
            nc.scalar.activation(
                v_aug[:, :, :, DK:DK + 1],
                cos_t[:, 0:SC * HPC].rearrange(
                    "p (a b o) -> p a b o", a=SC, b=HPC),
                mybir.ActivationFunctionType.Identity, bias=1.0, scale=0.0)

        # ---------------- phase 1: xT, q/k/v projections, RoPE ----------------
        for half in range(SH):
            sh = S // SH                      # 1024 s per half
            xt = sb_p1.tile([128, KC, sh], F32R, tag="xt")
            for sc in range(SC // SH):        # 8 chunks of 128 s
                s0 = half * sh + sc * 128
                xn = sb_p1.tile([128, D], F32, tag="x_nat", bufs=2)
                nc.sync.dma_start(xn[:], x_d.ap()[s0:s0 + 128, :])
                for dc in range(KC):
                    ptr = ps.tile([128, 128], F32, tag="ps_tr", bufs=3)
                    nc.tensor.transpose(ptr[:], xn[:, dc * 128:(dc + 1) * 128], ident[:])
                    with nc.allow_low_precision(reason="f32r matmul input"):
                        nc.vector.tensor_copy(
                            xt[:, dc, sc * 128:(sc + 1) * 128], ptr[:])

            # q/k transposed projections + RoPE
            for mat, (w_t, b_t, bs_t, dstT) in enumerate(
                    ((wq_t, bq_t, bqs_t, qT), (wk_t, bk_t, bks_t, kT))):
                for mc in range(MC):
                    for nt in range(sh // NT):    # 2 n-tiles per half
                        s0 = half * sh + nt * NT
                        pqk = ps.tile([128, NT], F32, tag="ps_qk", bufs=2)
                        for kc in range(KC):
                            nc.tensor.matmul(
                                pqk[:],
                                w_t[:, kc, mc * 128:(mc + 1) * 128],
                                xt[:, kc, nt * NT:(nt + 1) * NT],
                                start=(kc == 0), stop=(kc == KC - 1))
                        # RoPE (bias fused): dst = (pqk+b)*cos + rot(pqk+b)*sin_s
                        a_t = sb_p1.tile([128, NT], F32, tag="ropeA", bufs=2)
                        r_t = sb_p1.tile([128, NT], F32, tag="ropeR", bufs=2)
                        cs = cos_t[:, s0:s0 + NT]
                        ss = sin_t[:, s0:s0 + NT]
                        nc.vector.scalar_tensor_tensor(
                            a_t[:], pqk[:], b_t[:, mc:mc + 1], cs,
                            op0=mybir.AluOpType.add, op1=mybir.AluOpType.mult)
                        for g in range(4):
                            dst_p = g * 32
                            src_p = (g ^ 1) * 32
                            nc.vector.scalar_tensor_tensor(
                                r_t[dst_p:dst_p + 32, :],
                                pqk[src_p:src_p + 32, :],
                                bs_t[dst_p:dst_p + 32, mc:mc + 1],
                                ss[dst_p:dst_p + 32, :],
                                op0=mybir.AluOpType.add, op1=mybir.AluOpType.mult)
                        with nc.allow_low_precision(reason="f32r matmul input"):
                            nc.vector.tensor_add(
                                dstT[:, mc, s0:s0 + NT], a_t[:], r_t[:])

            # v natural projection, ones-augmented
            for sc in range(SC // SH):
                s0c = half * (SC // SH) + sc
                pv = ps.tile([128, DH], F32, tag="ps_v", bufs=2)
                for kc in range(KC):
                    nc.tensor.matmul(
                        pv[:], xt[:, kc, sc * 128:(sc + 1) * 128],
                        wv_t[:, kc, :], start=(kc == 0), stop=False)
                nc.tensor.matmul(
                    pv[:], ones_r[:, 0:128], bv_t[:], start=False, stop=True)
                nc.scalar.copy(
                    v_aug[:, s0c, :, 0:DK],
                    pv[:].rearrange("p (h d) -> p h d", h=HPC))

    # ---------------- phase 2: attention per head pair ----------------
    with (
        tc.tile_pool(name="sb_p2", bufs=1) as sb_p2,
        tc.tile_pool(name="ps2", bufs=1, space="PSUM") as ps,
    ):
        for pc in range(MC):            # head pair = (2pc, 2pc+1) local
            for qt in range(NQT):
                q0 = qt * NT
                pa0 = ps.tile([DK + 1, NT], F32, tag="ps_av0", bufs=1)
                pa1 = ps.tile([DK + 1, NT], F32, tag="ps_av1", bufs=1)
                for kc in range(SC):
                    k0 = kc * 128
                    ps0 = ps.tile([128, NT], F32, tag="ps_sc0", bufs=2)
                    ps1 = ps.tile([128, NT], F32, tag="ps_sc1", bufs=2)
                    nc.tensor.matmul(
                        ps0[:], kT[0:64, pc, k0:k0 + 128], qT[0:64, pc, q0:q0 + NT],
                        start=True, stop=True, tile_position=(0, 0))
                    nc.tensor.matmul(
                        ps1[:], kT[64:128, pc, k0:k0 + 128], qT[64:128, pc, q0:q0 + NT],
                        start=True, stop=True, tile_position=(64, 0))
                    e0 = sb_p2.tile([128, NT], F32R, tag="exp0", bufs=3)
                    e1 = sb_p2.tile([128, NT], F32R, tag="exp1", bufs=3)
                    with nc.allow_low_precision(reason="f32r exp"):
                        nc.scalar.activation(
                            e0[:], ps0[:], mybir.ActivationFunctionType.Exp,
                            scale=0.125)
                        nc.scalar.activation(
                            e1[:], ps1[:], mybir.ActivationFunctionType.Exp,
                            scale=0.125)
                    nc.tensor.matmul(
                        pa0[:], v_aug[:, kc, 2 * pc, :], e0[:],
                        start=(kc == 0), stop=(kc == SC - 1))
                    nc.tensor.matmul(
                        pa1[:], v_aug[:, kc, 2 * pc + 1, :], e1[:],
                        start=(kc == 0), stop=(kc == SC - 1))
                # normalize: ctx_h = pa_h[0:64] / Z_h  (Z in row 64)
                for h, pa in ((0, pa0), (1, pa1)):
                    rz = sb_p2.tile([1, NT], F32R, tag=f"recip{h}", bufs=2)
                    with nc.allow_low_precision(reason="f32r recip"):
                        nc.vector.reciprocal(rz[0:1, :], pa[DK:DK + 1, :])
                    pb = ps.tile([64, NT], F32, tag=f"ps_bc{h}", bufs=1)
                    nc.tensor.matmul(pb[:], ones_r[:, 0:64], rz[:],
                                     start=True, stop=True)
                    bc = sb_p2.tile([64, NT], F32, tag=f"bc{h}", bufs=2)
                    nc.scalar.copy(bc[:], pb[:])
                    with nc.allow_low_precision(reason="f32r ctx"):
                        nc.vector.tensor_mul(
                            ctx[64 * h:64 * h + 64, pc, q0:q0 + NT],
                            pa[0:DK, :], bc[:])

    # ---------------- phase 3: output projection ----------------
    with (
        tc.tile_pool(name="sb_p3", bufs=1) as sb_p3,
        tc.tile_pool(name="ps3", bufs=1, space="PSUM") as ps,
    ):
        for mc_o in range(D // 128):       # 8 output chunks
            for st in range(NQT):
                po = ps.tile([128, NT], F32, tag="ps_o", bufs=3)
                for c in range(MC):
                    nc.tensor.matmul(
                        po[:], wo_t[:, c, mc_o * 128:(mc_o + 1) * 128],
                        ctx[:, c, st * NT:(st + 1) * NT],
                        start=(c == 0), stop=(c == MC - 1))
                osb = sb_p3.tile([128, NT], F32, tag="osb", bufs=3)
                nc.vector.tensor_copy(osb[:], po[:])
                nc.sync.dma_start(
                    out_d.ap()[mc_o * 128:(mc_o + 1) * 128, st * NT:(st + 1) * NT],
                    osb[:])


def build_nc(reps=1):
    nc = bass.Bass("TRN2", target_bir_lowering=False, debug=False)
    tens = {
        "x_b": nc.dram_tensor("x_b", [S, D], F32, kind="ExternalInput"),
        "wq": nc.dram_tensor("wq", [D, DH], F32, kind="ExternalInput"),
        "wk": nc.dram_tensor("wk", [D, DH], F32, kind="ExternalInput"),
        "wv": nc.dram_tensor("wv", [D, DH], F32, kind="ExternalInput"),
        "wo": nc.dram_tensor("wo", [DH, D], F32, kind="ExternalInput"),
        "bq": nc.dram_tensor("bq", [DH], F32, kind="ExternalInput"),
        "bk": nc.dram_tensor("bk", [DH], F32, kind="ExternalInput"),
        "bv": nc.dram_tensor("bv", [1, DH], F32, kind="ExternalInput"),
        "bq_sh": nc.dram_tensor("bq_sh", [DH], F32, kind="ExternalInput"),
        "bk_sh": nc.dram_tensor("bk_sh", [DH], F32, kind="ExternalInput"),
        "cos_t": nc.dram_tensor("cos_t", [128, S], F32, kind="ExternalInput"),
        "sin_s": nc.dram_tensor("sin_s", [128, S], F32, kind="ExternalInput"),
        "ones": nc.dram_tensor("ones", [1, 128], F32, kind="ExternalInput"),
        "outT": nc.dram_tensor("outT", [D, S], F32, kind="ExternalOutput"),
    }
    with TileContext(nc) as tc:
        with tc.tile_pool(name="sb_main", bufs=1) as sb_main:
            for _ in range(reps):
                _build_core_body(nc, tens, sb_main, tc)
    _split_multi_waits(nc)
    return nc


_NC_CACHE = {}

_idx = np.arange(DH)
_SHIFT_PERM = (_idx // 128) * 128 + ((_idx % 128) ^ 32)


def _rope_tables():
    inv_freq = 1.0 / (10000.0 ** (np.arange(0, DK, 2, dtype=np.float64) / DK))
    pos = np.arange(S, dtype=np.float64)
    freqs = pos[:, None] * inv_freq[None, :]          # [S, 32]
    p = np.arange(128)
    cos = np.cos(freqs[:, p % 32]).T.astype(np.float32)       # [128, S]
    sgn = np.where((p % 64) < 32, -1.0, 1.0)
    sin = (np.sin(freqs[:, p % 32]) * sgn[None, :]).T.astype(np.float32)
    return np.ascontiguousarray(cos), np.ascontiguousarray(sin)


def kernel(x, Wq, bq, Wk, bk, Wv, bv, Wo, bo, _reps=1):
    x, Wq, bq, Wk, bk = (np.asarray(a, np.float32) for a in (x, Wq, bq, Wk, bk))
    Wv, bv, Wo, bo = (np.asarray(a, np.float32) for a in (Wv, bv, Wo, bo))

    if _reps not in _NC_CACHE:
        _NC_CACHE[_reps] = build_nc(_reps)
    nc = _NC_CACHE[_reps]

    cos_t, sin_s = _rope_tables()
    ones = np.ones((1, 128), np.float32)

    in_maps = []
    for core in range(8):
        b, c = core // TP, core % TP
        sl = slice(c * DH, (c + 1) * DH)
        in_maps.append({
            "x_b": np.ascontiguousarray(x[b]),
            "wq": np.ascontiguousarray(Wq[:, sl]),
            "wk": np.ascontiguousarray(Wk[:, sl]),
            "wv": np.ascontiguousarray(Wv[:, sl]),
            "wo": np.ascontiguousarray(Wo[sl, :]),
            "bq": np.ascontiguousarray(bq[sl]),
            "bk": np.ascontiguousarray(bk[sl]),
            "bv": np.ascontiguousarray(bv[sl]).reshape(1, DH),
            "bq_sh": np.ascontiguousarray(bq[sl][_SHIFT_PERM]),
            "bk_sh": np.ascontiguousarray(bk[sl][_SHIFT_PERM]),
            "cos_t": cos_t,
            "sin_s": sin_s,
            "ones": ones,
        })

    res = run_bass_kernel_spmd(nc, in_maps, list(range(8)))
    out = np.zeros((B, S, D), np.float32)
    for core in range(8):
        out[core // TP] += res.results[core]["outT"].T
    out += bo[None, None, :]
    return out
